# revision 29
# baseline (speedup 1.0000x reference)
"""Windowed (block-local) multi-head attention on 8 Trainium2 NeuronCores.

Reference computation (fp32):
    x:[B=2, T=8192, C=1024], w_qkv:[3C, C], w_out:[C, C]
    per window of W=512 rows: qkv projection, per-head (H=16, D=64)
    softmax(q k^T / 8) v, then output projection.

Sharding: the 32 (B*nW) independent windows are split 4-per-core
(sequence parallel, zero communication). Weights are replicated.

Host-side prep (free — not on the device clock): shard, transpose to
"contraction dim on partitions" layouts, cast to bf16.  Weight DRAM
layouts are blocked to match SBUF destinations so every DMA moves
long contiguous per-partition runs.

Device dataflow per window (all matmuls bf16, fp32 PSUM accumulate):
  qkT  [f, w] = w_qk @ x^T      (16 f-chunks x 8 k-chunks, N=512)
      q blocks land whole; k blocks are split per head into a kT tile
      whose other 64 rows stay zero, so the scores matmul runs at
      K=128 like every other matmul (the PE pays ~100ns to switch
      contraction size 64<->128; zero rows stream for free)
  v    [w, f]  = x @ w_v^T       (4 w-chunks x 2 o-tiles x 8 k-chunks)
  per head h:
    scoresT[j, i] = k_h^T q_h    (4 j-chunks, K=128 zero-padded, N=512)
    expT = exp(0.125 * scoresT)  (ACT, PSUM->SBUF bf16)
    out2T_aug[0:64,i], s[i] = [v_h | 1]^T @ expT  (augmented-V: softmax
        denominator falls out of row 64 of the same accumulating matmul)
    s row -> partition-0 tile (scalar) -> GPSIMD partition_broadcast
    -> DVE reciprocal_approx_fast in place (~0.7us, free-size driven)
    -> one DVE multiply does normalize + fp32->bf16 cast from PSUM
  y    [w, o] = out2 @ w_out^T  (accumulate 8 c-chunks) -> DMA fp32

Scheduling: a single software-pipelined stream.  Window w's attention
heads are interleaved with window w+1's QK/V projection matmuls AND
window w-1's y-projection matmuls (deferred one window).  The last
window's projection tail is carried into its own attention phase so
the PE stays busy end to end.  Startup streams just-in-time: every
weight f-block and x k-chunk is its own tile (consumers wait only on
their own DMA), triggers ride the sync+gpsimd queues only (the scalar
queue must stay clear for the first PSUM-draining copies), ordered so
the first qk chain's inputs land first.
"""

import os

import numpy as np
import ml_dtypes

import concourse.bass as bass
import concourse.tile as tile
import concourse.mybir as mybir
from concourse import bacc
from concourse.bass_utils import run_bass_kernel_spmd

BF16 = mybir.dt.bfloat16
F32 = mybir.dt.float32

B, T, C = 2, 8192, 1024
H, D, W = 16, 64, 512
NW = T // W          # 16 windows per batch element
NCORES = 8
NWPC = B * NW // NCORES  # 4 windows per core
CC = C // 128        # 8 contraction chunks
P = 128

_cache = {}


def _build_kernel(reps=1):
    nc = bacc.Bacc("TRN2", target_bir_lowering=False, debug=False)

    xT_d = nc.dram_tensor("xT", [NWPC, P, CC, W], BF16, kind="ExternalInput").ap()
    # fc-blocked: [P, fc, cc, 128] so one f-block DMA is a contiguous
    # 2KB-per-partition run on both sides
    wqkT_d = nc.dram_tensor("wqkT", [P, 16, CC, P], BF16, kind="ExternalInput").ap()
    wvT_d = nc.dram_tensor("wvT", [P, 2, CC, W], BF16, kind="ExternalInput").ap()
    woutT_d = nc.dram_tensor("woutT", [P, 2, CC, W], BF16, kind="ExternalInput").ap()
    y_d = nc.dram_tensor("y", [NWPC, W, C], F32, kind="ExternalOutput").ap()

    with tile.TileContext(nc) as tc:
        with (
            tc.tile_pool(name="wconst", bufs=1) as wpool,
            tc.tile_pool(name="xt", bufs=2) as xpool,
            tc.tile_pool(name="qk", bufs=2) as qkpool,
            tc.tile_pool(name="vp", bufs=2) as vpool,
            tc.tile_pool(name="expp", bufs=4) as epool,
            tc.tile_pool(name="yin", bufs=2) as ypool,
            tc.tile_pool(name="sbc", bufs=4) as spool,
            tc.tile_pool(name="rbc", bufs=3) as rpool,
            tc.tile_pool(name="ps_mm", bufs=2, space="PSUM") as ps_mm,
            tc.tile_pool(name="ps_sc", bufs=2, space="PSUM") as ps_sc,
            tc.tile_pool(name="ps_av", bufs=2, space="PSUM") as ps_av,
        ):
            wqk_sbs = [
                wpool.tile([P, CC, P], BF16, name=f"wqk{fc}") for fc in range(16)
            ]
            wv_sbs = [
                wpool.tile([P, CC, W], BF16, name=f"wv{ot}") for ot in range(2)
            ]
            wout_sbs = [
                wpool.tile([P, CC, W], BF16, name=f"wout{ot}") for ot in range(2)
            ]

            if reps > 1:
                rep_ctx = tc.For_i(0, reps, 1)
                rep_ctx.__enter__()

            # per-window state
            xts = [None] * NWPC
            qkTs = [None] * NWPC
            kTps = [None] * NWPC
            vs = [None] * NWPC
            yins = [None] * NWPC

            # ---- startup DMAs ----
            # Every weight f-block and x k-chunk is its own tile so a
            # consumer waits only on its own DMA.  Triggers (each costs
            # ~0.7us of sequencer time) are spread across three queues
            # in critical-first order: the first qk chain's inputs
            # (x0 chunks + early f-blocks) land first.
            def xtile(win, cc):
                return xpool.tile([P, W], BF16, tag=f"x{cc}",
                                  name=f"xt{win}_{cc}")

            xts[0] = [xtile(0, cc) for cc in range(CC)]
            xts[1] = [xtile(1, cc) for cc in range(CC)]
            # sync + gpsimd only: the scalar queue must stay DMA-free so
            # the first qk copies are not stuck behind DMA triggers.
            # Hand-ordered so the first chain's inputs (fc0, x0 chunks)
            # are the first triggers on each queue.
            xq = lambda cc: (xts[0][cc], xT_d[0, :, cc, :])  # noqa: E731
            wq = lambda fc: (wqk_sbs[fc], wqkT_d[:, fc])  # noqa: E731
            startup = [xq(0), xq(1), wq(0), xq(2), xq(3), wq(1), xq(4),
                       xq(5), wq(2), xq(6), xq(7), wq(3)]
            startup += [wq(fc) for fc in range(4, 16)]
            startup += [(wv_sbs[0], wvT_d[:, 0]), (wv_sbs[1], wvT_d[:, 1])]
            startup += [(xts[1][cc], xT_d[1, :, cc, :]) for cc in range(CC)]
            startup += [(wout_sbs[0], woutT_d[:, 0]), (wout_sbs[1], woutT_d[:, 1])]
            qs = [nc.sync, nc.gpsimd]
            for i, (dst, srcd) in enumerate(startup):
                qs[i % 2].dma_start(dst[:], srcd)

            def load_x_cc(win, cc):
                if xts[win] is None:
                    xts[win] = [xtile(win, c) for c in range(CC)]
                nc.sync.dma_start(xts[win][cc][:], xT_d[win, :, cc, :])

            def qk_piece(win, fc):
                # one f-chunk of the QK projection.  q blocks land in
                # qT whole; k blocks are split per head into kTp with
                # the other head's 64 rows left zero, so the scores
                # matmul runs at K=128 like everything else (avoids the
                # PE's K-reconfigure penalty; zero rows are free).
                if qkTs[win] is None:
                    qkTs[win] = qkpool.tile([P, 8, W], BF16, tag="qT",
                                            name=f"qT{win}")
                    kTps[win] = qkpool.tile([P, H, W], BF16, tag="kTp",
                                            name=f"kTp{win}")
                    if win < 2:
                        # zero the pad rows once per ring slot; the
                        # copies below never touch them so the zeros
                        # survive ring reuse
                        for h in range(H):
                            lo = 0 if h % 2 else D
                            nc.gpsimd.memset(
                                kTps[win][lo:lo + D, h, :], 0.0
                            )
                qkT = qkTs[win]
                xt = xts[win]
                ps = ps_mm.tile([P, W], F32, tag="mm", name="ps")
                for cc in range(CC):
                    nc.tensor.matmul(
                        ps[:],
                        wqk_sbs[fc][:, cc, :],
                        xt[cc][:],
                        start=(cc == 0),
                        stop=(cc == CC - 1),
                    )
                if fc < 8:
                    if fc % 2 == 0:
                        nc.scalar.copy(qkT[:, fc, :], ps[:])
                    else:
                        nc.vector.tensor_copy(qkT[:, fc, :], ps[:])
                else:
                    k2 = 2 * (fc - 8)
                    nc.scalar.copy(kTps[win][0:D, k2, :], ps[0:D, :])
                    nc.vector.tensor_copy(
                        kTps[win][D:P, k2 + 1, :], ps[D:P, :]
                    )

            def v_piece(win, i):
                wc, ot = divmod(i, 2)
                if vs[win] is None:
                    vs[win] = vpool.tile([P, 4, H, D + 1], BF16, tag="v",
                                         name=f"v{win}")
                    nc.vector.memset(vs[win][:, :, :, D:D + 1], 1.0)
                v_sb = vs[win]
                xt = xts[win]
                ps = ps_mm.tile([P, W], F32, tag="mm", name="ps")
                for cc in range(CC):
                    nc.tensor.matmul(
                        ps[:],
                        xt[cc][:, wc * P:(wc + 1) * P],
                        wv_sbs[ot][:, cc, :],
                        start=(cc == 0),
                        stop=(cc == CC - 1),
                    )
                nc.vector.tensor_copy(
                    v_sb[:, wc, ot * 8:(ot + 1) * 8, 0:D],
                    ps.rearrange("p (h d) -> p h d", d=D),
                )

            def proj_pieces(win, interleaved):
                qk = lambda fc: (lambda: qk_piece(win, fc))  # noqa: E731
                vv = lambda i: (lambda: v_piece(win, i))  # noqa: E731
                if not interleaved:
                    return [qk(fc) for fc in range(16)] + [vv(i) for i in range(8)]
                # ordered so attention head h's inputs appear early:
                # head h needs fc h//2, fc 8+h//2, and v ot=h//8
                ops = [qk(0), qk(8), vv(0), vv(2), vv(4), vv(6)]
                for k in range(1, 8):
                    ops += [qk(k), qk(8 + k)]
                ops += [vv(1), vv(3), vv(5), vv(7)]
                return ops

            def attn_scores(win, h):
                qT = qkTs[win][:, h // 2, :]
                kT = kTps[win][:, h, :]
                expT = epool.tile([P, 4, W], BF16, tag="expT", name="expT")
                for half in range(2):
                    ps_s = ps_sc.tile([P, 2, W], F32, tag="sc", name="ps_s")
                    for j in range(2):
                        jc = half * 2 + j
                        nc.tensor.matmul(
                            ps_s[:, j, :],
                            kT[:, jc * P:(jc + 1) * P],
                            qT,
                            start=True,
                            stop=True,
                        )
                    nc.scalar.activation(
                        expT[:, half * 2:half * 2 + 2, :], ps_s[:],
                        mybir.ActivationFunctionType.Exp,
                        scale=0.125,
                    )
                return expT

            def attn_av(win, h, expT):
                v_sb = vs[win]
                ps_o = ps_av.tile([D + 1, W], F32, tag="av", name="ps_o")
                for jc in range(4):
                    nc.tensor.matmul(
                        ps_o[:],
                        v_sb[:, jc, h, :],
                        expT[:, jc, :],
                        start=(jc == 0),
                        stop=(jc == 3),
                    )
                s_one = spool.tile([1, W], F32, tag="sone", bufs=3, name="s_one")
                nc.scalar.copy(s_one[:], ps_o[D:D + 1, :])
                r_bf = spool.tile([1, W], BF16, tag="rbf1", bufs=3, name="r_bf")
                nc.vector.reciprocal_approx_fast(s_one[:], s_one[:])
                nc.vector.tensor_copy(r_bf[:], s_one[:])
                rbc = rpool.tile([D, W], BF16, tag="rbc", name="rbc")
                nc.gpsimd.partition_broadcast(rbc[:], r_bf[:])
                y_in = yins[win]
                nc.vector.tensor_mul(
                    y_in[(h % 2) * D:(h % 2) * D + D, h // 2, :],
                    ps_o[0:D, :],
                    rbc[:],
                )

            def y_piece(win, i, halves=1):
                wc, ot = divmod(i, 2)
                y_in = yins[win]
                hw = W // halves
                for hf in range(halves):
                    ps = ps_mm.tile([P, W], F32, tag="mm", name="ps")
                    for cc in range(CC):
                        nc.tensor.matmul(
                            ps[:, 0:hw],
                            y_in[:, cc, wc * P:(wc + 1) * P],
                            wout_sbs[ot][:, cc, hf * hw:(hf + 1) * hw],
                            start=(cc == 0),
                            stop=(cc == CC - 1),
                        )
                    y_sb = spool.tile([P, W], F32, tag="ysb", name="y_sb")
                    nc.vector.tensor_copy(y_sb[:, 0:hw], ps[:, 0:hw])
                    nc.sync.dma_start(
                        y_d[win, wc * P:(wc + 1) * P,
                            ot * W + hf * hw:ot * W + (hf + 1) * hw],
                        y_sb[:, 0:hw],
                    )

            # ---------- pipelined schedule ----------
            for op in proj_pieces(0, interleaved=False):
                op()

            proj3 = proj_pieces(3, interleaved=True)
            ypq = {w: [lambda w=w, i=i: y_piece(w, i) for i in range(8)]
                   for w in range(NWPC)}
            ypq[3] = ypq[3][:]
            fq = {
                0: proj_pieces(1, interleaved=True),
                1: proj_pieces(2, interleaved=True) + ypq[0],
                2: proj3[:16] + ypq[1],
                3: proj3[16:] + ypq[2],
            }

            for win in range(NWPC):
                yins[win] = ypool.tile([P, CC, W], BF16, tag="yin", name=f"yin{win}")
                fillers = fq[win]
                done = 0
                for h in range(H):
                    expT = attn_scores(win, h)
                    if win + 2 < NWPC and h < CC:
                        load_x_cc(win + 2, h)
                    want = (h + 1) * len(fillers) // H
                    while done < want:
                        fillers[done]()
                        done += 1
                    attn_av(win, h, expT)

            # epilogue: last window's output projection
            for op in ypq[3]:
                op()

            if reps > 1:
                rep_ctx.__exit__(None, None, None)
    nc.compile()
    return nc


def _prep_inputs(x, w_qkv, w_out):
    bf16 = ml_dtypes.bfloat16
    # x -> per-window transposed [32, 128, 8, 512] (c on partitions)
    xw = np.asarray(x, np.float32).reshape(B * NW, W, C)
    xT = xw.transpose(0, 2, 1).reshape(B * NW, CC, P, W)
    xT = np.ascontiguousarray(xT.transpose(0, 2, 1, 3)).astype(bf16)

    # wqk: [2C, C] -> [128ci, 16fc, 8cc, 128m];  elem = w[fc*128+m, cc*128+ci]
    a = np.asarray(w_qkv[:2 * C], np.float32).reshape(16, P, CC, P)
    wqkT = np.ascontiguousarray(a.transpose(3, 0, 2, 1)).astype(bf16)

    def prep_w2(wt):  # [C, C] -> [128ci, 2ot, 8cc, 512n]
        b = np.asarray(wt, np.float32).reshape(2, W, CC, P)
        return np.ascontiguousarray(b.transpose(3, 0, 2, 1)).astype(bf16)

    wvT = prep_w2(w_qkv[2 * C:])
    woutT = prep_w2(w_out)
    return xT, wqkT, wvT, woutT


def kernel(x, w_qkv, w_out):
    if "nc" not in _cache:
        _cache["nc"] = _build_kernel()
    nc = _cache["nc"]

    xT, wqkT, wvT, woutT = _prep_inputs(x, w_qkv, w_out)
    in_maps = [
        {
            "xT": np.ascontiguousarray(xT[c * NWPC:(c + 1) * NWPC]),
            "wqkT": wqkT,
            "wvT": wvT,
            "woutT": woutT,
        }
        for c in range(NCORES)
    ]
    trace = os.environ.get("KERNEL_TRACE", "0") == "1"
    res = run_bass_kernel_spmd(nc, in_maps, list(range(NCORES)), trace=trace)
    if trace:
        _cache["last_results"] = res

    y = np.concatenate([np.asarray(res.results[c]["y"]) for c in range(NCORES)],
                       axis=0)
    return y.reshape(B, T, C).astype(np.float32)


# revision 30
# speedup vs baseline: 1.0049x; 1.0049x over previous
"""Windowed (block-local) multi-head attention on 8 Trainium2 NeuronCores.

Reference computation (fp32):
    x:[B=2, T=8192, C=1024], w_qkv:[3C, C], w_out:[C, C]
    per window of W=512 rows: qkv projection, per-head (H=16, D=64)
    softmax(q k^T / 8) v, then output projection.

Sharding: the 32 (B*nW) independent windows are split 4-per-core
(sequence parallel, zero communication). Weights are replicated.

Host-side prep (free — not on the device clock): shard, transpose to
"contraction dim on partitions" layouts, cast to bf16.  Weight DRAM
layouts are blocked to match SBUF destinations so every DMA moves
long contiguous per-partition runs.

Device dataflow per window (all matmuls bf16, fp32 PSUM accumulate):
  qkT  [f, w] = w_qk @ x^T      (16 f-chunks x 8 k-chunks, N=512)
      q blocks land whole; k blocks are split per head into a kT tile
      whose other 64 rows stay zero, so the scores matmul runs at
      K=128 like every other matmul (the PE pays ~100ns to switch
      contraction size 64<->128; zero rows stream for free)
  v    [w, f]  = x @ w_v^T       (4 w-chunks x 2 o-tiles x 8 k-chunks)
  per head h:
    scoresT[j, i] = k_h^T q_h    (4 j-chunks, K=128 zero-padded, N=512)
    expT = exp(0.125 * scoresT)  (ACT, PSUM->SBUF bf16)
    out2T_aug[0:64,i], s[i] = [v_h | 1]^T @ expT  (augmented-V: softmax
        denominator falls out of row 64 of the same accumulating matmul)
    s row -> partition-0 tile (scalar) -> GPSIMD partition_broadcast
    -> DVE reciprocal_approx_fast in place (~0.7us, free-size driven)
    -> one DVE multiply does normalize + fp32->bf16 cast from PSUM
  y    [w, o] = out2 @ w_out^T  (accumulate 8 c-chunks) -> DMA fp32

Scheduling: a single software-pipelined stream.  Window w's attention
heads are interleaved with window w+1's QK/V projection matmuls AND
window w-1's y-projection matmuls (deferred one window).  The last
window's projection tail is carried into its own attention phase so
the PE stays busy end to end.  Startup streams just-in-time: every
weight f-block and x k-chunk is its own tile (consumers wait only on
their own DMA), triggers ride the sync+gpsimd queues only (the scalar
queue must stay clear for the first PSUM-draining copies), ordered so
the first qk chain's inputs land first.
"""

import os

import numpy as np
import ml_dtypes

import concourse.bass as bass
import concourse.tile as tile
import concourse.mybir as mybir
from concourse import bacc
from concourse.bass_utils import run_bass_kernel_spmd

BF16 = mybir.dt.bfloat16
F32 = mybir.dt.float32

B, T, C = 2, 8192, 1024
H, D, W = 16, 64, 512
NW = T // W          # 16 windows per batch element
NCORES = 8
NWPC = B * NW // NCORES  # 4 windows per core
CC = C // 128        # 8 contraction chunks
P = 128

_cache = {}


def _build_kernel(reps=1):
    nc = bacc.Bacc("TRN2", target_bir_lowering=False, debug=False)

    xT_d = nc.dram_tensor("xT", [NWPC, P, CC, W], BF16, kind="ExternalInput").ap()
    # fc-blocked: [P, fc, cc, 128] so one f-block DMA is a contiguous
    # 2KB-per-partition run on both sides
    wqkT_d = nc.dram_tensor("wqkT", [P, 16, CC, P], BF16, kind="ExternalInput").ap()
    wvT_d = nc.dram_tensor("wvT", [P, 2, CC, W], BF16, kind="ExternalInput").ap()
    woutT_d = nc.dram_tensor("woutT", [P, 2, CC, W], BF16, kind="ExternalInput").ap()
    y_d = nc.dram_tensor("y", [NWPC, W, C], F32, kind="ExternalOutput").ap()

    with tile.TileContext(nc) as tc:
        with (
            tc.tile_pool(name="wconst", bufs=1) as wpool,
            tc.tile_pool(name="xt", bufs=2) as xpool,
            tc.tile_pool(name="qk", bufs=2) as qkpool,
            tc.tile_pool(name="vp", bufs=2) as vpool,
            tc.tile_pool(name="expp", bufs=4) as epool,
            tc.tile_pool(name="yin", bufs=2) as ypool,
            tc.tile_pool(name="sbc", bufs=4) as spool,
            tc.tile_pool(name="rbc", bufs=3) as rpool,
            tc.tile_pool(name="ps_mm", bufs=2, space="PSUM") as ps_mm,
            tc.tile_pool(name="ps_sc", bufs=2, space="PSUM") as ps_sc,
            tc.tile_pool(name="ps_av", bufs=2, space="PSUM") as ps_av,
        ):
            wqk_sbs = [
                wpool.tile([P, CC, P], BF16, name=f"wqk{fc}") for fc in range(16)
            ]
            wv_sbs = [
                wpool.tile([P, CC, W], BF16, name=f"wv{ot}") for ot in range(2)
            ]
            wout_sbs = [
                wpool.tile([P, CC, W], BF16, name=f"wout{ot}") for ot in range(2)
            ]

            if reps > 1:
                rep_ctx = tc.For_i(0, reps, 1)
                rep_ctx.__enter__()

            # per-window state
            xts = [None] * NWPC
            qkTs = [None] * NWPC
            kTps = [None] * NWPC
            vs = [None] * NWPC
            yins = [None] * NWPC

            # ---- startup DMAs ----
            # Every weight f-block and x k-chunk is its own tile so a
            # consumer waits only on its own DMA.  Triggers (each costs
            # ~0.7us of sequencer time) are spread across three queues
            # in critical-first order: the first qk chain's inputs
            # (x0 chunks + early f-blocks) land first.
            def xtile(win, cc):
                return xpool.tile([P, W], BF16, tag=f"x{cc}",
                                  name=f"xt{win}_{cc}")

            xts[0] = [xtile(0, cc) for cc in range(CC)]
            xts[1] = [xtile(1, cc) for cc in range(CC)]
            # sync + gpsimd only: the scalar queue must stay DMA-free so
            # the first qk copies are not stuck behind DMA triggers.
            # Hand-ordered so the first chain's inputs (fc0, x0 chunks)
            # are the first triggers on each queue.
            xq = lambda cc: (xts[0][cc], xT_d[0, :, cc, :])  # noqa: E731
            wq = lambda fc: (wqk_sbs[fc], wqkT_d[:, fc])  # noqa: E731
            startup = [xq(0), xq(1), wq(0), xq(2), xq(3), wq(1), xq(4),
                       xq(5), wq(2), xq(6), xq(7), wq(3)]
            startup += [wq(fc) for fc in range(4, 16)]
            startup += [(wv_sbs[0], wvT_d[:, 0]), (wv_sbs[1], wvT_d[:, 1])]
            startup += [(xts[1][cc], xT_d[1, :, cc, :]) for cc in range(CC)]
            startup += [(wout_sbs[0], woutT_d[:, 0]), (wout_sbs[1], woutT_d[:, 1])]
            qs = [nc.sync, nc.gpsimd]
            for i, (dst, srcd) in enumerate(startup):
                qs[i % 2].dma_start(dst[:], srcd)

            def load_x_cc(win, cc):
                if xts[win] is None:
                    xts[win] = [xtile(win, c) for c in range(CC)]
                nc.sync.dma_start(xts[win][cc][:], xT_d[win, :, cc, :])

            def qk_piece(win, fc):
                # one f-chunk of the QK projection.  q blocks land in
                # qT whole; k blocks are split per head into kTp with
                # the other head's 64 rows left zero, so the scores
                # matmul runs at K=128 like everything else (avoids the
                # PE's K-reconfigure penalty; zero rows are free).
                if qkTs[win] is None:
                    qkTs[win] = qkpool.tile([P, 8, W], BF16, tag="qT",
                                            name=f"qT{win}")
                    kTps[win] = qkpool.tile([P, H, W], BF16, tag="kTp",
                                            name=f"kTp{win}")
                    if win < 2:
                        # zero the pad rows once per ring slot; the
                        # copies below never touch them so the zeros
                        # survive ring reuse
                        for h in range(H):
                            lo = 0 if h % 2 else D
                            nc.gpsimd.memset(
                                kTps[win][lo:lo + D, h, :], 0.0
                            )
                qkT = qkTs[win]
                xt = xts[win]
                ps = ps_mm.tile([P, W], F32, tag="mm", name="ps")
                for cc in range(CC):
                    nc.tensor.matmul(
                        ps[:],
                        wqk_sbs[fc][:, cc, :],
                        xt[cc][:],
                        start=(cc == 0),
                        stop=(cc == CC - 1),
                    )
                if fc < 8:
                    nc.scalar.copy(qkT[0:D, fc, :], ps[0:D, :])
                    nc.vector.tensor_copy(qkT[D:P, fc, :], ps[D:P, :])
                else:
                    k2 = 2 * (fc - 8)
                    nc.scalar.copy(kTps[win][0:D, k2, :], ps[0:D, :])
                    nc.vector.tensor_copy(
                        kTps[win][D:P, k2 + 1, :], ps[D:P, :]
                    )

            def v_piece(win, i):
                wc, ot = divmod(i, 2)
                if vs[win] is None:
                    vs[win] = vpool.tile([P, 4, H, D + 1], BF16, tag="v",
                                         name=f"v{win}")
                    nc.vector.memset(vs[win][:, :, :, D:D + 1], 1.0)
                v_sb = vs[win]
                xt = xts[win]
                ps = ps_mm.tile([P, W], F32, tag="mm", name="ps")
                for cc in range(CC):
                    nc.tensor.matmul(
                        ps[:],
                        xt[cc][:, wc * P:(wc + 1) * P],
                        wv_sbs[ot][:, cc, :],
                        start=(cc == 0),
                        stop=(cc == CC - 1),
                    )
                nc.vector.tensor_copy(
                    v_sb[:, wc, ot * 8:(ot + 1) * 8, 0:D],
                    ps.rearrange("p (h d) -> p h d", d=D),
                )

            def proj_pieces(win, interleaved):
                qk = lambda fc: (lambda: qk_piece(win, fc))  # noqa: E731
                vv = lambda i: (lambda: v_piece(win, i))  # noqa: E731
                if not interleaved:
                    return [qk(fc) for fc in range(16)] + [vv(i) for i in range(8)]
                # ordered so attention head h's inputs appear early:
                # head h needs fc h//2, fc 8+h//2, and v ot=h//8
                ops = [qk(0), qk(8), vv(0), vv(2), vv(4), vv(6)]
                for k in range(1, 8):
                    ops += [qk(k), qk(8 + k)]
                ops += [vv(1), vv(3), vv(5), vv(7)]
                return ops

            def attn_scores(win, h):
                qT = qkTs[win][:, h // 2, :]
                kT = kTps[win][:, h, :]
                expT = epool.tile([P, 4, W], BF16, tag="expT", name="expT")
                for half in range(2):
                    ps_s = ps_sc.tile([P, 2, W], F32, tag="sc", name="ps_s")
                    for j in range(2):
                        jc = half * 2 + j
                        nc.tensor.matmul(
                            ps_s[:, j, :],
                            kT[:, jc * P:(jc + 1) * P],
                            qT,
                            start=True,
                            stop=True,
                        )
                    nc.scalar.activation(
                        expT[:, half * 2:half * 2 + 2, :], ps_s[:],
                        mybir.ActivationFunctionType.Exp,
                        scale=0.125,
                    )
                return expT

            def attn_av(win, h, expT):
                v_sb = vs[win]
                ps_o = ps_av.tile([D + 1, W], F32, tag="av", name="ps_o")
                for jc in range(4):
                    nc.tensor.matmul(
                        ps_o[:],
                        v_sb[:, jc, h, :],
                        expT[:, jc, :],
                        start=(jc == 0),
                        stop=(jc == 3),
                    )
                s_one = spool.tile([1, W], F32, tag="sone", bufs=3, name="s_one")
                nc.scalar.copy(s_one[:], ps_o[D:D + 1, :])
                rbc = rpool.tile([D, W], F32, tag="rbc", name="rbc")
                nc.gpsimd.partition_broadcast(rbc[:], s_one[:])
                nc.vector.reciprocal_approx_fast(rbc[:], rbc[:])
                y_in = yins[win]
                nc.vector.tensor_mul(
                    y_in[(h % 2) * D:(h % 2) * D + D, h // 2, :],
                    ps_o[0:D, :],
                    rbc[:],
                )

            def y_piece(win, i, halves=1):
                wc, ot = divmod(i, 2)
                y_in = yins[win]
                hw = W // halves
                for hf in range(halves):
                    ps = ps_mm.tile([P, W], F32, tag="mm", name="ps")
                    for cc in range(CC):
                        nc.tensor.matmul(
                            ps[:, 0:hw],
                            y_in[:, cc, wc * P:(wc + 1) * P],
                            wout_sbs[ot][:, cc, hf * hw:(hf + 1) * hw],
                            start=(cc == 0),
                            stop=(cc == CC - 1),
                        )
                    y_sb = spool.tile([P, W], F32, tag="ysb", name="y_sb")
                    nc.vector.tensor_copy(y_sb[:, 0:hw], ps[:, 0:hw])
                    nc.sync.dma_start(
                        y_d[win, wc * P:(wc + 1) * P,
                            ot * W + hf * hw:ot * W + (hf + 1) * hw],
                        y_sb[:, 0:hw],
                    )

            # ---------- pipelined schedule ----------
            for op in proj_pieces(0, interleaved=False):
                op()

            proj3 = proj_pieces(3, interleaved=True)
            ypq = {w: [lambda w=w, i=i: y_piece(w, i) for i in range(8)]
                   for w in range(NWPC)}
            ypq[3] = ypq[3][:]
            fq = {
                0: proj_pieces(1, interleaved=True),
                1: proj_pieces(2, interleaved=True) + ypq[0],
                2: proj3[:16] + ypq[1],
                3: proj3[16:] + ypq[2],
            }

            for win in range(NWPC):
                yins[win] = ypool.tile([P, CC, W], BF16, tag="yin", name=f"yin{win}")
                fillers = fq[win]
                done = 0
                for h in range(H):
                    expT = attn_scores(win, h)
                    if win + 2 < NWPC and h < CC:
                        load_x_cc(win + 2, h)
                    want = (h + 1) * len(fillers) // H
                    while done < want:
                        fillers[done]()
                        done += 1
                    attn_av(win, h, expT)

            # epilogue: last window's output projection
            for op in ypq[3]:
                op()

            if reps > 1:
                rep_ctx.__exit__(None, None, None)
    nc.compile()
    return nc


def _prep_inputs(x, w_qkv, w_out):
    bf16 = ml_dtypes.bfloat16
    # x -> per-window transposed [32, 128, 8, 512] (c on partitions)
    xw = np.asarray(x, np.float32).reshape(B * NW, W, C)
    xT = xw.transpose(0, 2, 1).reshape(B * NW, CC, P, W)
    xT = np.ascontiguousarray(xT.transpose(0, 2, 1, 3)).astype(bf16)

    # wqk: [2C, C] -> [128ci, 16fc, 8cc, 128m];  elem = w[fc*128+m, cc*128+ci]
    a = np.asarray(w_qkv[:2 * C], np.float32).reshape(16, P, CC, P)
    wqkT = np.ascontiguousarray(a.transpose(3, 0, 2, 1)).astype(bf16)

    def prep_w2(wt):  # [C, C] -> [128ci, 2ot, 8cc, 512n]
        b = np.asarray(wt, np.float32).reshape(2, W, CC, P)
        return np.ascontiguousarray(b.transpose(3, 0, 2, 1)).astype(bf16)

    wvT = prep_w2(w_qkv[2 * C:])
    woutT = prep_w2(w_out)
    return xT, wqkT, wvT, woutT


def kernel(x, w_qkv, w_out):
    if "nc" not in _cache:
        _cache["nc"] = _build_kernel()
    nc = _cache["nc"]

    xT, wqkT, wvT, woutT = _prep_inputs(x, w_qkv, w_out)
    in_maps = [
        {
            "xT": np.ascontiguousarray(xT[c * NWPC:(c + 1) * NWPC]),
            "wqkT": wqkT,
            "wvT": wvT,
            "woutT": woutT,
        }
        for c in range(NCORES)
    ]
    trace = os.environ.get("KERNEL_TRACE", "0") == "1"
    res = run_bass_kernel_spmd(nc, in_maps, list(range(NCORES)), trace=trace)
    if trace:
        _cache["last_results"] = res

    y = np.concatenate([np.asarray(res.results[c]["y"]) for c in range(NCORES)],
                       axis=0)
    return y.reshape(B, T, C).astype(np.float32)
